# revision 23
# baseline (speedup 1.0000x reference)
import os
import sys

for _p in ("/opt/trn_rl_repo", "/root/.axon_site/_ro/trn_rl_repo"):
    if os.path.isdir(_p) and _p not in sys.path:
        sys.path.insert(0, _p)

import numpy as np
import concourse.bacc as bacc
import concourse.mybir as mybir
import concourse.tile as tile
from concourse import bass_utils

B, N, T, F = 8, 128, 2048, 32
L, H = 5, 64

FP32 = mybir.dt.float32
FP16 = mybir.dt.float16
FP32R = mybir.dt.float32r

HALO = 4          # max_lag - 1
CHUNK = 16        # t-steps per output chunk
NCHUNKS = T // CHUNK  # 128
NG = NCHUNKS // 2     # 64 groups (2 chunks each)

# graduated x tiles: small first tiles so the PE can start early
TILE_T = [32, 96, 128] + [256] * 7
TILE_SLICES = [2, 2, 2] + [8] * 7
TILE_START = [sum(TILE_T[:i]) for i in range(len(TILE_T))]
assert sum(TILE_T) == T

Y_CHUNK_FREE = CHUNK * H       # 1024
WARMUP = 36

_CACHE = {}
LAST_RESULTS = None


def _build_nc():
    nc = bacc.Bacc("TRN2", target_bir_lowering=False, debug=False)
    x_d = nc.dram_tensor("x", (N, T * F), FP16, kind="ExternalInput").ap()
    at_d = nc.dram_tensor("at", (N, L * N), FP16, kind="ExternalInput").ap()
    wd_d = nc.dram_tensor("wd", (128, 256), FP16, kind="ExternalInput").ap()
    bvec_d = nc.dram_tensor("bvec", (128, 1), FP32, kind="ExternalInput").ap()
    y_d = nc.dram_tensor("y", (N, T * H), FP16, kind="ExternalOutput").ap()

    if os.environ.get("SIM_NOGELU"):
        gelu = mybir.ActivationFunctionType.Identity
    else:
        gelu = mybir.ActivationFunctionType.Gelu

    # chunk -> tile index
    tile_of_chunk = []
    for ti, (s, sz) in enumerate(zip(TILE_START, TILE_T)):
        tile_of_chunk += [ti] * (sz // CHUNK)
    first_chunk_of_tile = {}
    for g, ti in enumerate(tile_of_chunk):
        first_chunk_of_tile.setdefault(ti, g)
    # when chunk g starts, kick off the load of tile(ti+1)
    load_at_chunk = {}
    for ti, g in first_chunk_of_tile.items():
        if ti >= 1 and ti + 1 < len(TILE_T):
            load_at_chunk[g] = ti + 1

    with tile.TileContext(nc) as tc:
        with (
            tc.tile_pool(name="sb", bufs=1) as sb,
            tc.tile_pool(name="ps", bufs=2, space="PSUM") as ps,
        ):
            at_sb = sb.tile((N, L * N), FP16, tag="at")
            wd_sb = sb.tile((128, 256), FP16, tag="wd")
            bvec_sb = sb.tile((128, 1), FP32, tag="bvec")

            x_tiles = {}
            paggs = {}
            t16s = {}

            def emit_xload(ti):
                sz = TILE_T[ti]
                s = TILE_START[ti]
                nsl = TILE_SLICES[ti]
                free = (sz + HALO) * F
                x_tile = sb.tile((N, free), FP16, tag="x", bufs=2, name="xt",
                                 padded_shape=[N, (256 + HALO) * F])
                x_tiles[ti] = x_tile
                if ti == 0:
                    nc.any.memset(x_tile[:, 0 : HALO * F], 0.0)
                    src = x_d[:, 0 : sz * F]
                    sl = sz * F // nsl
                    for q in range(nsl):
                        nc.sync.dma_start(
                            out=x_tile[:, HALO * F + q * sl : HALO * F + (q + 1) * sl],
                            in_=src[:, q * sl : (q + 1) * sl],
                        )
                else:
                    src = x_d[:, (s - HALO) * F : (s + sz) * F]
                    sl = free // nsl
                    assert free % nsl == 0
                    for q in range(nsl):
                        nc.sync.dma_start(
                            out=x_tile[:, q * sl : (q + 1) * sl],
                            in_=src[:, q * sl : (q + 1) * sl],
                        )

            # ---- program order: DMAs first so the measured window starts
            # at the first DMA issue, not at an early memset ----
            nc.sync.dma_start(out=at_sb, in_=at_d)
            emit_xload(0)
            emit_xload(1)
            nc.sync.dma_start(out=wd_sb, in_=wd_d)
            nc.sync.dma_start(out=bvec_sb, in_=bvec_d)

            # tiny activation with no DMA deps: pulls the gelu ACT_TABLE_LOAD
            # into the preamble window instead of behind the first s2
            warm_sb = sb.tile((1, 2), FP32, tag="warm")
            nc.vector.memset(warm_sb, 0.0)
            nc.scalar.activation(warm_sb, warm_sb, func=gelu)
            # dummy matmuls on zeroed SBUF during the x-DMA wait: PE activity
            # ramps the clock (HAM) before the real stream starts
            pewarm_sb = sb.tile((N, 128), FP16, tag="pewarm")
            nc.vector.memset(pewarm_sb, 0.0)
            psum_warm = ps.tile((N, 1024), FP32, tag="py", bufs=3, name="psum_warm")
            for _w in range(WARMUP):
                nc.tensor.matmul(
                    psum_warm[:, 0:128],
                    pewarm_sb,
                    pewarm_sb,
                    start=True,
                    stop=True,
                )

            def emit_s1(g):
                # per-chunk PSUM accumulator (1 bank) so py can be 3 deep
                paggs[g] = ps.tile((N, 512), FP32, tag="pagg", name="pagg")
                psum_agg = paggs[g]
                ti = tile_of_chunk[g]
                x_tile = x_tiles[ti]
                t0 = g * CHUNK
                base = (t0 - TILE_START[ti] + HALO) * F
                out = psum_agg
                for lag in range(L):
                    off = base - lag * F
                    nc.tensor.matmul(
                        out,
                        at_sb[:, lag * N : (lag + 1) * N],
                        x_tile[:, off : off + 512],
                        start=(lag == 0),
                        stop=(lag == L - 1),
                    )

            def emit_trh(g):
                # per-chunk transpose+cast on DVE
                psum_agg = paggs.pop(g)
                trh = sb.tile((N, 512), FP32, tag="trh", bufs=4, name="trh")
                nc.vector.transpose(trh, psum_agg)
                t16h = sb.tile((N, 512), FP16, tag="t16h", bufs=6, name="t16h")
                nc.vector.tensor_copy(t16h, trh)
                t16s[g] = t16h

            def emit_s2(g):
                rhs = t16s.pop(g)
                psum_y = ps.tile((N, Y_CHUNK_FREE), FP32, tag="py", bufs=3, name="py")
                for r in range(2):
                    nc.tensor.matmul(
                        psum_y[:, r * 512 : (r + 1) * 512],
                        wd_sb[:, r * 128 : (r + 1) * 128],
                        rhs,
                        start=True,
                        stop=True,
                    )
                sbuf_y = sb.tile((N, Y_CHUNK_FREE), FP16, tag="y", bufs=8, name="yt")
                nc.scalar.activation(
                    sbuf_y,
                    psum_y,
                    func=gelu,
                    bias=bvec_sb,
                )
                nc.sync.dma_start(
                    out=y_d[:, g * Y_CHUNK_FREE : (g + 1) * Y_CHUNK_FREE],
                    in_=sbuf_y,
                )

            def emit_s2_last(g):
                # final chunk: quarter-granularity (8 t) transpose/cast/s2/
                # act/DMA so the drain chain after the last s1 is short
                pagg = paggs.pop(g)
                psum_y = ps.tile((N, Y_CHUNK_FREE), FP32, tag="py", bufs=3, name="py")
                sbuf_y = sb.tile((N, Y_CHUNK_FREE), FP16, tag="y", bufs=8, name="yt")
                pin = psum_y.rearrange("p (r c) -> p r c", r=2)
                yc = y_d[:, g * Y_CHUNK_FREE : (g + 1) * Y_CHUNK_FREE].rearrange(
                    "p (r c) -> p r c", r=2
                )
                for q in range(2):
                    trq = sb.tile((N, 256), FP32, tag="trq", bufs=2, name="trq")
                    nc.vector.transpose(trq, pagg[:, q * 256 : (q + 1) * 256])
                    cq = sb.tile((N, 256), FP16, tag="cq", bufs=2, name="cq")
                    nc.vector.tensor_copy(cq, trq)
                    for r in range(2):
                        # q=0 zeroes the whole 2KB region (start=True); q=1
                        # accumulates into the still-pending-zero half
                        nc.tensor.matmul(
                            psum_y[:, r * 512 + q * 256 : r * 512 + (q + 1) * 256],
                            wd_sb[:, r * 128 : (r + 1) * 128],
                            cq,
                            start=(q == 0),
                            stop=True,
                            skip_group_check=(q == 1),
                        )
                    nc.scalar.activation(
                        sbuf_y[:, q * 512 : (q + 1) * 512],
                        pin[:, :, q * 256 : (q + 1) * 256],
                        func=gelu,
                        bias=bvec_sb,
                    )
                    nc.sync.dma_start(
                        out=yc[:, :, q * 256 : (q + 1) * 256],
                        in_=sbuf_y[:, q * 512 : (q + 1) * 512],
                    )

            # steady state: s1+tr per chunk | s2 pair (j-2); tail s2
            # emission compressed so only the final group's activations
            # drain after the last s1
            for j in range(NG - 2):
                g0 = 2 * j
                emit_s1(g0)
                emit_trh(g0)
                if g0 in load_at_chunk:
                    emit_xload(load_at_chunk[g0])
                emit_s1(g0 + 1)
                if g0 + 1 in load_at_chunk:
                    emit_xload(load_at_chunk[g0 + 1])
                emit_trh(g0 + 1)
                if j >= 2:
                    emit_s2(2 * (j - 2))
                    emit_s2(2 * (j - 2) + 1)
            # j = NG-2: s1 pair, pairs (NG-4) and (NG-3) of s2
            emit_s1(2 * (NG - 2))
            emit_trh(2 * (NG - 2))
            emit_s2(2 * (NG - 4))
            emit_s2(2 * (NG - 4) + 1)
            emit_s1(2 * (NG - 2) + 1)
            emit_trh(2 * (NG - 2) + 1)
            emit_s2(2 * (NG - 3))
            emit_s2(2 * (NG - 3) + 1)
            # j = NG-1: last s1 pair; pair (NG-2) between the two s1 chunks
            emit_s1(2 * (NG - 1))
            emit_trh(2 * (NG - 1))
            emit_s2(2 * (NG - 2))
            emit_s2(2 * (NG - 2) + 1)
            emit_s1(2 * (NG - 1) + 1)
            # drain: final group only; last chunk at quarter granularity
            emit_s2(2 * (NG - 1))
            emit_s2_last(2 * (NG - 1) + 1)
    nc.compile()
    return nc


def _host_inputs(x, A_list, W, b):
    # wd holds the two S2 lhsT matrices side by side:
    # lhsT_r[32*g + f, 64*d + h] = W[h, f] if g == 2*r + d else 0
    wd = np.zeros((128, 256), np.float16)
    wt = W.T.astype(np.float16)  # [f, h] = [32, 64]
    for r in range(2):
        for d in range(2):
            g = 2 * r + d
            wd[32 * g : 32 * g + 32, 128 * r + 64 * d : 128 * r + 64 * d + 64] = wt
    bvec = np.ascontiguousarray(np.tile(b, 2)[:, None].astype(np.float32))

    in_maps = []
    for c in range(x.shape[0]):
        in_maps.append(
            {
                "x": x[c].reshape(N, T * F).astype(np.float16),
                "at": np.ascontiguousarray(
                    A_list[c].transpose(2, 0, 1).reshape(N, L * N)
                ).astype(np.float16),
                "wd": wd,
                "bvec": bvec,
            }
        )
    return in_maps


def _decode_y(arr):
    # arr: [128, T*H] partitions p = 64*d + h;
    # free col = g*1024 + r*512 + tl*32 + il;
    # value = z[i = 64*r + 32*d + il, t = 16*g + tl, h]
    arr6 = arr.reshape(2, 64, T // CHUNK, 2, CHUNK, 32)
    yb = (
        np.transpose(arr6, (3, 0, 5, 2, 4, 1))
        .reshape(N, T, H)
        .astype(np.float32)
    )
    return yb


def kernel(x, A_list, W, b):
    global LAST_RESULTS
    x = np.asarray(x, np.float32)
    A_list = np.asarray(A_list, np.float32)
    W = np.asarray(W, np.float32)
    b = np.asarray(b, np.float32)

    if "nc" not in _CACHE:
        _CACHE["nc"] = _build_nc()
    nc = _CACHE["nc"]

    in_maps = _host_inputs(x, A_list, W, b)

    trace = bool(os.environ.get("KERNEL_TRACE"))
    res = bass_utils.run_bass_kernel_spmd(
        nc, in_maps, core_ids=list(range(B)), trace=trace
    )
    LAST_RESULTS = res
    outs = []
    for c in range(x.shape[0]):
        arr = np.asarray(res.results[c]["y"])
        outs.append(_decode_y(arr))
    return np.stack(outs)
